# revision 22
# baseline (speedup 1.0000x reference)
"""Trainium2 Bass kernel for nn_DecoderFactoredLSTM (v3: col-tiled bf16 pipeline).

Factored-LSTM decoder:
  emb = B_w[captions]                                   [B,T,E] -> tokens [T*B, E]
  u   = emb @ (V^T S^T U^T) + bias                      [T*B, 4H]   (gate pre-activations)
  recurrence over T=40 steps (LSTM, no tanh on c for h)
  out = hiddens @ C_w^T + C_b                           [T*B, V]

Sharding: recurrence + pre-projections replicated on all 8 cores; the
vocab projection (dominant FLOPs) sharded 8-way over vocab columns.

v3 = v2 (fused bf16 pipeline, SBUF-resident u/hT, interleaved fillers)
plus PE column-tiling: batch is 64 so gate/u matmuls only fill half the
128-wide stationary array; pairs of matmuls at tile_position (0,0) and
(0,64) run CONCURRENTLY, nearly halving recurrence+u PE time. Outputs
land stacked on psum partitions p = b + 64*v (v = h-half of the gate),
which also makes every elementwise op a full 128-partition [128,512] op.

Column layout (gate-major): col = g*1024 + 512*v + h'. W/M rhs chunk
(g,v) = cols [(2g+v)*512, ...). hT kept as pairT[buf][ki, ko, par*64+b]
(par = step parity in the m-tile); h-tile [b+64v, (q,h'')] transposes
q-slices into k-tiles ko = q + 4v.
"""

import sys
from contextlib import ExitStack

if "/opt/trn_rl_repo" not in sys.path:
    sys.path.insert(0, "/opt/trn_rl_repo")

import numpy as np

import concourse.bass as bass
import concourse.mybir as mybir
import concourse.tile as tile
from concourse import bacc
from concourse.bass import ts
from concourse.bass_utils import run_bass_kernel_spmd
from concourse.masks import make_identity

B, T, E, H, F, V = 64, 40, 512, 1024, 512, 32000
NCORES = 8
VS = V // NCORES  # 4000
TOK = T * B  # 2560
MT = TOK // 128  # 20
NV = VS // 8  # 500
F32 = mybir.dt.float32
F32R = mybir.dt.float32r
BF16 = mybir.dt.bfloat16
SIG = mybir.ActivationFunctionType.Sigmoid
TANH = mybir.ActivationFunctionType.Tanh
COPY = mybir.ActivationFunctionType.Copy

GATE_ORDER = [3, 0, 1, 2]  # ctilde first (tanh feeds the c-chain), then i, f, o


def _build():
    nc = bacc.Bacc(None, target_bir_lowering=False, debug=False)

    with tile.TileContext(nc) as tc:
        cap_d = nc.declare_dram_parameter("cap", [TOK, 1], mybir.dt.int32, isOutput=False)
        Bw_d = nc.declare_dram_parameter("Bw", [V, E], F32, isOutput=False)
        Vg_d = nc.declare_dram_parameter("Vg", [4, F, E], F32R, isOutput=False)
        SgT_d = nc.declare_dram_parameter("SgT", [4, F, F], F32R, isOutput=False)
        UgT_d = nc.declare_dram_parameter("UgT", [4, F, H], F32R, isOutput=False)
        W_d = nc.declare_dram_parameter("Wmov", [H, 4 * H], BF16, isOutput=False)
        ub_d = nc.declare_dram_parameter("ubias2", [128, 2 * H], BF16, isOutput=False)
        CT_d = nc.declare_dram_parameter("CT", [H, VS], BF16, isOutput=False)
        out_d = nc.declare_dram_parameter("out", [TOK, VS], BF16, isOutput=True)

        embT_d = nc.dram_tensor("embT", [4, 128, TOK], BF16)  # [e_outer, e_inner, tok]

        with ExitStack() as stack:
            pers = stack.enter_context(tc.tile_pool(name="pers", bufs=1))
            idb = pers.tile([128, 128], BF16, tag="idb")
            make_identity(nc, idb)
            # token indices first on the sync queue so gathers start early
            idx_all = pers.tile([128, MT], mybir.dt.int32, tag="idx")
            nc.sync.dma_start(
                idx_all[:], cap_d[:].rearrange("(m p) o -> p (m o)", p=128)
            )
            wsbA = pers.tile([128, 4, 4 * H], BF16, tag="wsbA")
            nc.sync.dma_start(
                wsbA[:], W_d[0 : H // 2, :].rearrange("(ko ki) n -> ki ko n", ki=128)
            )
            wsbB = pers.tile([128, 4, 4 * H], BF16, tag="wsbB")
            nc.sync.dma_start(
                wsbB[:], W_d[H // 2 :, :].rearrange("(ko ki) n -> ki ko n", ki=128)
            )
            ubias = pers.tile([128, 2 * H], BF16, tag="ubias")
            nc.sync.dma_start(ubias[:], ub_d[:])
            mcat = pers.tile([128, 4, 4 * H], BF16, tag="mcat")
            # u2_sb[mparity][stepparity]: stacked u' [b+64v, (g, h')]
            u2_sb = [
                [
                    pers.tile([128, 2 * H], BF16, tag=f"u{i}{s}", name=f"u{i}{s}")
                    for s in range(2)
                ]
                for i in range(2)
            ]
            pairT = [
                pers.tile([128, 8, 128], BF16, tag=f"pairT{i}", name=f"pairT{i}")
                for i in range(2)
            ]
            c_sb = pers.tile([128, 512], F32, tag="c")

            # ================= prologue: M = V^T S^T U^T, and embT =========
            with (
                tc.tile_pool(name="ph2a", bufs=2) as ph2a,
                tc.tile_pool(name="ph2u", bufs=2) as ph2u,
                tc.tile_pool(name="ph2b", bufs=1) as ph2b,
                tc.tile_pool(name="ph2ps", bufs=2, space="PSUM") as ph2ps,
                tc.tile_pool(name="ph1b", bufs=2) as ph1b,
                tc.tile_pool(name="ph1ps", bufs=2, space="PSUM") as ph1ps,
            ):

                def m_gate(g):
                    vg = ph2a.tile([128, 4, E], F32R, tag="vg")
                    nc.scalar.dma_start(
                        vg[:], Vg_d[g].rearrange("(ko ki) e -> ki ko e", ki=128)
                    )
                    sgT = ph2a.tile([128, 4, F], F32R, tag="sgT")
                    nc.scalar.dma_start(
                        sgT[:], SgT_d[g].rearrange("(ko ki) f -> ki ko f", ki=128)
                    )
                    ugT = ph2u.tile([128, 4, H], F32R, tag="ugT")
                    nc.scalar.dma_start(
                        ugT[:], UgT_d[g].rearrange("(ko ki) f -> ki ko f", ki=128)
                    )
                    # PT[f', e] = sum_f S[f',f] V[f,e]
                    pt = ph2b.tile([128, 4, E], F32R, tag="pt")
                    for fp in range(4):
                        ps = ph2ps.tile([128, E], F32, tag="mp")
                        for k in range(4):
                            nc.tensor.matmul(
                                ps[:],
                                lhsT=sgT[:, k, ts(fp, 128)],
                                rhs=vg[:, k, :],
                                start=(k == 0),
                                stop=(k == 3),
                            )
                        nc.vector.tensor_copy(pt[:, fp, :], ps[:])
                    # M[e, col], col = g*1024 + nh*512 + h'
                    for e_t in range(4):
                        for nh in range(2):
                            ps2 = ph2ps.tile([128, 512], F32, tag="mp")
                            for k in range(4):
                                nc.tensor.matmul(
                                    ps2[:],
                                    lhsT=pt[:, k, ts(e_t, 128)],
                                    rhs=ugT[:, k, ts(nh, 512)],
                                    start=(k == 0),
                                    stop=(k == 3),
                                )
                            nc.vector.tensor_copy(
                                mcat[:, e_t, g * 1024 + nh * 512 : g * 1024 + (nh + 1) * 512],
                                ps2[:],
                            )

                def ph1_m(m):
                    g_t = ph1b.tile([128, E], F32, tag="gt")
                    nc.gpsimd.indirect_dma_start(
                        out=g_t[:],
                        out_offset=None,
                        in_=Bw_d[:],
                        in_offset=bass.IndirectOffsetOnAxis(
                            ap=idx_all[:, m : m + 1], axis=0
                        ),
                    )
                    gb = ph1b.tile([128, E], BF16, tag="gb")
                    nc.vector.tensor_copy(gb[:], g_t[:])
                    stg = ph1b.tile([128, 4, 128], BF16, tag="stg")
                    for e in range(4):
                        tp = ph1ps.tile([128, 128], BF16, tag="tp1")
                        nc.tensor.transpose(tp[:], gb[:, ts(e, 128)], idb[:])
                        nc.vector.tensor_copy(stg[:, e, :], tp[:])
                    nc.sync.dma_start(
                        embT_d[:].rearrange("e ki t -> ki e t")[:, :, ts(m, 128)],
                        stg[:],
                    )

                for g in range(4):
                    m_gate(g)
                    for m in range(5 * g, 5 * g + 5):
                        ph1_m(m)

            # ================= main pipeline pools =========================
            ltp = stack.enter_context(tc.tile_pool(name="ltp", bufs=2))
            gsp = stack.enter_context(tc.tile_pool(name="gsp", bufs=2))
            sigp = stack.enter_context(tc.tile_pool(name="sigp", bufs=5))
            itp = stack.enter_context(tc.tile_pool(name="itp", bufs=2))
            fcp = stack.enter_context(tc.tile_pool(name="fcp", bufs=2))
            htp = stack.enter_context(tc.tile_pool(name="htp", bufs=2))
            pevp = stack.enter_context(tc.tile_pool(name="pevp", bufs=2))
            recp = stack.enter_context(tc.tile_pool(name="recp", bufs=3, space="PSUM"))
            up = stack.enter_context(tc.tile_pool(name="up", bufs=2, space="PSUM"))
            vp = stack.enter_context(tc.tile_pool(name="vp", bufs=2, space="PSUM"))
            tpp = stack.enter_context(tc.tile_pool(name="tpp", bufs=1, space="PSUM"))

            def load_lt(mt):
                lts = []
                for k in range(4):
                    lt = ltp.tile([128, 128], BF16, tag=f"lt{k}")
                    nc.scalar.dma_start(lt[:], embT_d[k, :, ts(mt, 128)])
                    lts.append(lt)
                return lts

            def u_steps(lts, mt, steps, gates=range(4)):
                """Produce stacked u' for steps (parities) of m-tile mt."""
                for s in steps:
                    for g in gates:
                        ps = up.tile([128, 512], F32, tag="up")
                        for k in range(4):
                            lhsT = lts[k][:, s * 64 : (s + 1) * 64]
                            nc.tensor.matmul(
                                ps[0:64, :],
                                lhsT=lhsT,
                                rhs=mcat[:, k, ts(2 * g, 512)],
                                start=(k == 0),
                                stop=(k == 3),
                                tile_position=(0, 0),
                            )
                            nc.tensor.matmul(
                                ps[64:128, :],
                                lhsT=lhsT,
                                rhs=mcat[:, k, ts(2 * g + 1, 512)],
                                start=(k == 0),
                                stop=(k == 3),
                                tile_position=(0, 64),
                            )
                        nc.vector.tensor_add(
                            u2_sb[mt % 2][s][:, ts(g, 512)], ps[:], ubias[:, ts(g, 512)]
                        )

            def rec_step(t):
                """One LSTM step: col-tiled gate matmuls + stacked elementwise."""
                mb = (t // 2) % 2
                upar = t % 2
                pmb = ((t - 1) // 2) % 2
                ppar = (t - 1) % 2
                sg = {}
                for g in GATE_ORDER:
                    if t == 0:
                        src = u2_sb[0][0][:, ts(g, 512)]
                    else:
                        ps = recp.tile([128, 512], F32, tag="rp")
                        for k in range(8):
                            wsbX = wsbA if k < 4 else wsbB
                            lhsT = pairT[pmb][:, k, ppar * 64 : (ppar + 1) * 64]
                            nc.tensor.matmul(
                                ps[0:64, :],
                                lhsT=lhsT,
                                rhs=wsbX[:, k % 4, ts(2 * g, 512)],
                                start=(k == 0),
                                stop=(k == 7),
                                tile_position=(0, 0),
                            )
                            nc.tensor.matmul(
                                ps[64:128, :],
                                lhsT=lhsT,
                                rhs=wsbX[:, k % 4, ts(2 * g + 1, 512)],
                                start=(k == 0),
                                stop=(k == 7),
                                tile_position=(0, 64),
                            )
                        gs = gsp.tile([128, 512], BF16, tag="gs")
                        nc.vector.tensor_add(
                            gs[:], ps[:], u2_sb[mb][upar][:, ts(g, 512)]
                        )
                        src = gs[:]
                    sgt = sigp.tile([128, 512], BF16, tag="sig")
                    nc.scalar.activation(sgt[:], src, TANH if g == 3 else SIG)
                    sg[g] = sgt
                    if g == 1:  # ctilde, i, f done -> start the c-chain early
                        if t == 0:
                            nc.vector.tensor_mul(c_sb[:], sg[0][:], sg[3][:])
                        else:
                            it = itp.tile([128, 512], BF16, tag="it")
                            nc.vector.tensor_mul(it[:], sg[0][:], sg[3][:])
                            fc = fcp.tile([128, 512], BF16, tag="fc")
                            nc.vector.tensor_mul(fc[:], sg[1][:], c_sb[:])
                            nc.vector.tensor_add(c_sb[:], fc[:], it[:])
                ht = htp.tile([128, 512], BF16, tag="ht")
                nc.vector.tensor_mul(ht[:], sg[2][:], c_sb[:])
                return ht

            def transposes(t, ht):
                mb = (t // 2) % 2
                par = t % 2
                tp = tpp.tile([128, 4, 128], BF16, tag="tp")
                for q in range(4):
                    nc.tensor.transpose(tp[:, q, :], ht[:, ts(q, 128)], idb[:])
                # 2 strided copies (on ACT - DVE is chain-congested):
                # pairT[:, q+4l, par*64+b] <- tp[:, q, l*64+b]
                for l in range(2):
                    nc.scalar.activation(
                        pairT[mb][:, 4 * l : 4 * l + 4, par * 64 : (par + 1) * 64],
                        tp[:, :, ts(l, 64)],
                        COPY,
                    )

            def voc_chunks(vm, chunks):
                for n in chunks:
                    ps = vp.tile([128, NV], F32, tag="vpp")
                    for k in range(8):
                        ctX = ctA if k < 4 else ctB
                        nc.tensor.matmul(
                            ps[:],
                            lhsT=pairT[vm % 2][:, k, :],
                            rhs=ctX[:, k % 4, ts(n, NV)],
                            start=(k == 0),
                            stop=(k == 7),
                        )
                    pev = pevp.tile([128, NV], BF16, tag="pev")
                    nc.scalar.activation(pev[:], ps[:], COPY)
                    nc.gpsimd.dma_start(out_d[ts(vm, 128), ts(n, NV)], pev[:])

            # u(0) before the big late-weight DMAs so it isn't queued
            # behind 8MB of C^T transfer
            lts = load_lt(0)
            u_steps(lts, 0, [0, 1])

            # late vocab weights (transfers overlap the first iterations)
            late = stack.enter_context(tc.tile_pool(name="late", bufs=1))
            ctA = late.tile([128, 4, VS], BF16, tag="ctA")
            nc.sync.dma_start(
                ctA[:], CT_d[0 : H // 2, :].rearrange("(ko ki) n -> ki ko n", ki=128)
            )
            ctB = late.tile([128, 4, VS], BF16, tag="ctB")
            nc.sync.dma_start(
                ctB[:], CT_d[H // 2 :, :].rearrange("(ko ki) n -> ki ko n", ki=128)
            )

            for m in range(MT):
                lts = load_lt(m + 1) if m + 1 < MT else None
                ht = rec_step(2 * m)
                if lts is not None:
                    u_steps(lts, m + 1, [0], range(0, 2))
                transposes(2 * m, ht)
                if lts is not None:
                    u_steps(lts, m + 1, [0], range(2, 4))
                if m >= 1:
                    voc_chunks(m - 1, range(4, 8))
                ht = rec_step(2 * m + 1)
                if lts is not None:
                    u_steps(lts, m + 1, [1], range(0, 2))
                transposes(2 * m + 1, ht)
                if lts is not None:
                    u_steps(lts, m + 1, [1], range(2, 4))
                voc_chunks(m, range(4))
            voc_chunks(MT - 1, range(4, 8))

    nc.compile()
    return nc


def kernel(**inputs):
    import ml_dtypes

    BF = ml_dtypes.bfloat16

    captions = np.asarray(inputs["captions"])
    B_w = np.asarray(inputs["B_w"], dtype=np.float32)
    V_w = np.asarray(inputs["V_w"], dtype=np.float32)
    V_b = np.asarray(inputs["V_b"], dtype=np.float32)
    S_w = np.asarray(inputs["S_w"], dtype=np.float32)
    S_b = np.asarray(inputs["S_b"], dtype=np.float32)
    U_w = np.asarray(inputs["U_w"], dtype=np.float32)
    U_b = np.asarray(inputs["U_b"], dtype=np.float32)
    W_w = np.asarray(inputs["W_w"], dtype=np.float32)
    W_b = np.asarray(inputs["W_b"], dtype=np.float32)
    C_w = np.asarray(inputs["C_w"], dtype=np.float32)
    C_b = np.asarray(inputs["C_b"], dtype=np.float32)

    # --- host-side layout prep (weights only) ---
    cap = np.ascontiguousarray(captions.T.reshape(TOK, 1)).astype(np.int32)
    SgT = np.ascontiguousarray(S_w.transpose(0, 2, 1))
    UgT = np.ascontiguousarray(U_w.transpose(0, 2, 1))
    # gate-major columns: col = g*1024 + h
    Wmov = np.ascontiguousarray(W_w.transpose(2, 0, 1).reshape(H, 4 * H)).astype(BF)
    # gate bias chain, folded: ((V_b @ S^T + S_b) @ U^T + U_b) + W_b
    bs = np.einsum("gf,gof->go", V_b, S_w) + S_b
    bu = np.einsum("gf,ghf->gh", bs, U_w) + U_b
    gate_bias = (bu + W_b).reshape(4, 2, 512)
    # stacked bias: ub2[b + 64v, g*512 + h'] = gate_bias[g, v, h']
    ub2 = np.empty((128, 2 * H), np.float32)
    ub2[0:64, :] = np.broadcast_to(gate_bias[:, 0, :].reshape(2 * H), (64, 2 * H))
    ub2[64:128, :] = np.broadcast_to(gate_bias[:, 1, :].reshape(2 * H), (64, 2 * H))
    ub2 = np.ascontiguousarray(ub2).astype(BF)
    CT = np.ascontiguousarray(C_w.T)  # [H, V]

    nc = _build()

    in_maps = []
    for c in range(NCORES):
        in_maps.append(
            {
                "cap": cap,
                "Bw": B_w,
                "Vg": V_w,
                "SgT": SgT,
                "UgT": UgT,
                "Wmov": Wmov,
                "ubias2": ub2,
                "CT": np.ascontiguousarray(CT[:, c * VS : (c + 1) * VS]).astype(BF),
            }
        )

    global _last_in_maps
    _last_in_maps = in_maps

    res = run_bass_kernel_spmd(nc, in_maps, list(range(NCORES)))
    out = np.concatenate(
        [res.results[c]["out"].astype(np.float32) for c in range(NCORES)], axis=1
    )
    out += C_b[None, :]
    return out


_last_in_maps = None


# revision 27
# speedup vs baseline: 1.0394x; 1.0394x over previous
"""Trainium2 Bass kernel for nn_DecoderFactoredLSTM (v3: col-tiled bf16 pipeline).

Factored-LSTM decoder:
  emb = B_w[captions]                                   [B,T,E] -> tokens [T*B, E]
  u   = emb @ (V^T S^T U^T) + bias                      [T*B, 4H]   (gate pre-activations)
  recurrence over T=40 steps (LSTM, no tanh on c for h)
  out = hiddens @ C_w^T + C_b                           [T*B, V]

Sharding: recurrence + pre-projections replicated on all 8 cores; the
vocab projection (dominant FLOPs) sharded 8-way over vocab columns.

v3 = v2 (fused bf16 pipeline, SBUF-resident u/hT, interleaved fillers)
plus PE column-tiling: batch is 64 so gate/u matmuls only fill half the
128-wide stationary array; pairs of matmuls at tile_position (0,0) and
(0,64) run CONCURRENTLY, nearly halving recurrence+u PE time. Outputs
land stacked on psum partitions p = b + 64*v (v = h-half of the gate),
which also makes every elementwise op a full 128-partition [128,512] op.

Column layout (gate-major): col = g*1024 + 512*v + h'. W/M rhs chunk
(g,v) = cols [(2g+v)*512, ...). hT kept as pairT[buf][ki, ko, par*64+b]
(par = step parity in the m-tile); h-tile [b+64v, (q,h'')] transposes
q-slices into k-tiles ko = q + 4v.
"""

import sys
from contextlib import ExitStack

if "/opt/trn_rl_repo" not in sys.path:
    sys.path.insert(0, "/opt/trn_rl_repo")

import numpy as np

import concourse.bass as bass
import concourse.mybir as mybir
import concourse.tile as tile
from concourse import bacc
from concourse.bass import ts
from concourse.bass_utils import run_bass_kernel_spmd
from concourse.masks import make_identity

B, T, E, H, F, V = 64, 40, 512, 1024, 512, 32000
NCORES = 8
VS = V // NCORES  # 4000
TOK = T * B  # 2560
MT = TOK // 128  # 20
NV = VS // 8  # 500
F32 = mybir.dt.float32
F32R = mybir.dt.float32r
BF16 = mybir.dt.bfloat16
SIG = mybir.ActivationFunctionType.Sigmoid
TANH = mybir.ActivationFunctionType.Tanh
COPY = mybir.ActivationFunctionType.Copy

GATE_ORDER = [3, 0, 1, 2]  # ctilde first (tanh feeds the c-chain), then i, f, o


def _build():
    nc = bacc.Bacc(None, target_bir_lowering=False, debug=False)

    with tile.TileContext(nc) as tc:
        cap_d = nc.declare_dram_parameter("cap", [128, MT], mybir.dt.int32, isOutput=False)
        Bw_d = nc.declare_dram_parameter("Bw", [V, E], F32, isOutput=False)
        Vg_d = nc.declare_dram_parameter("Vg", [4, F, E], F32R, isOutput=False)
        SgT_d = nc.declare_dram_parameter("SgT", [4, F, F], F32R, isOutput=False)
        UgT_d = nc.declare_dram_parameter("UgT", [4, F, H], F32R, isOutput=False)
        W_d = nc.declare_dram_parameter("Wmov", [H, 4 * H], BF16, isOutput=False)
        ub_d = nc.declare_dram_parameter("ubias2", [128, 2 * H], BF16, isOutput=False)
        CT_d = nc.declare_dram_parameter("CT", [H, VS], BF16, isOutput=False)
        out_d = nc.declare_dram_parameter("out", [TOK, VS], BF16, isOutput=True)

        embT_d = nc.dram_tensor("embT", [4, 128, TOK], BF16)  # [e_outer, e_inner, tok]

        with ExitStack() as stack:
            pers = stack.enter_context(tc.tile_pool(name="pers", bufs=1))
            idb = pers.tile([128, 128], BF16, tag="idb")
            make_identity(nc, idb)
            # token indices first on the sync queue so gathers start early
            # (host pre-swizzles to [p, m] so the DMA is contiguous)
            idx_all = pers.tile([128, MT], mybir.dt.int32, tag="idx")
            nc.sync.dma_start(idx_all[:], cap_d[:])
            wsbA = pers.tile([128, 4, 4 * H], BF16, tag="wsbA")
            nc.sync.dma_start(
                wsbA[:], W_d[0 : H // 2, :].rearrange("(ko ki) n -> ki ko n", ki=128)
            )
            wsbB = pers.tile([128, 4, 4 * H], BF16, tag="wsbB")
            nc.sync.dma_start(
                wsbB[:], W_d[H // 2 :, :].rearrange("(ko ki) n -> ki ko n", ki=128)
            )
            ubias = pers.tile([128, 2 * H], BF16, tag="ubias")
            nc.sync.dma_start(ubias[:], ub_d[:])
            mcat = pers.tile([128, 4, 4 * H], BF16, tag="mcat")
            # u2_sb[mparity][stepparity]: stacked u' [b+64v, (g, h')]
            u2_sb = [
                [
                    pers.tile([128, 2 * H], BF16, tag=f"u{i}{s}", name=f"u{i}{s}")
                    for s in range(2)
                ]
                for i in range(2)
            ]
            pairT = [
                pers.tile([128, 8, 128], BF16, tag=f"pairT{i}", name=f"pairT{i}")
                for i in range(2)
            ]
            c_sb = pers.tile([128, 512], F32, tag="c")

            # ================= prologue: M = V^T S^T U^T, and embT =========
            with (
                tc.tile_pool(name="ph2a", bufs=2) as ph2a,
                tc.tile_pool(name="ph2u", bufs=2) as ph2u,
                tc.tile_pool(name="ph2b", bufs=1) as ph2b,
                tc.tile_pool(name="ph2ps", bufs=2, space="PSUM") as ph2ps,
                tc.tile_pool(name="ph1b", bufs=2) as ph1b,
                tc.tile_pool(name="ph1ps", bufs=2, space="PSUM") as ph1ps,
            ):

                def m_gate(g):
                    vg = ph2a.tile([128, 4, E], F32R, tag="vg")
                    nc.scalar.dma_start(
                        vg[:], Vg_d[g].rearrange("(ko ki) e -> ki ko e", ki=128)
                    )
                    sgT = ph2a.tile([128, 4, F], F32R, tag="sgT")
                    nc.scalar.dma_start(
                        sgT[:], SgT_d[g].rearrange("(ko ki) f -> ki ko f", ki=128)
                    )
                    ugT = ph2u.tile([128, 4, H], F32R, tag="ugT")
                    nc.scalar.dma_start(
                        ugT[:], UgT_d[g].rearrange("(ko ki) f -> ki ko f", ki=128)
                    )
                    # PT[f', e] = sum_f S[f',f] V[f,e]
                    pt = ph2b.tile([128, 4, E], F32R, tag="pt")
                    for fp in range(4):
                        ps = ph2ps.tile([128, E], F32, tag="mp")
                        for k in range(4):
                            nc.tensor.matmul(
                                ps[:],
                                lhsT=sgT[:, k, ts(fp, 128)],
                                rhs=vg[:, k, :],
                                start=(k == 0),
                                stop=(k == 3),
                            )
                        nc.vector.tensor_copy(pt[:, fp, :], ps[:])
                    # M[e, col], col = g*1024 + nh*512 + h'
                    for e_t in range(4):
                        for nh in range(2):
                            ps2 = ph2ps.tile([128, 512], F32, tag="mp")
                            for k in range(4):
                                nc.tensor.matmul(
                                    ps2[:],
                                    lhsT=pt[:, k, ts(e_t, 128)],
                                    rhs=ugT[:, k, ts(nh, 512)],
                                    start=(k == 0),
                                    stop=(k == 3),
                                )
                            nc.vector.tensor_copy(
                                mcat[:, e_t, g * 1024 + nh * 512 : g * 1024 + (nh + 1) * 512],
                                ps2[:],
                            )

                def ph1_m(m):
                    g_t = ph1b.tile([128, E], F32, tag="gt")
                    nc.gpsimd.indirect_dma_start(
                        out=g_t[:],
                        out_offset=None,
                        in_=Bw_d[:],
                        in_offset=bass.IndirectOffsetOnAxis(
                            ap=idx_all[:, m : m + 1], axis=0
                        ),
                    )
                    gb = ph1b.tile([128, E], BF16, tag="gb")
                    nc.vector.tensor_copy(gb[:], g_t[:])
                    stg = ph1b.tile([128, 4, 128], BF16, tag="stg")
                    for e in range(4):
                        tp = ph1ps.tile([128, 128], BF16, tag="tp1")
                        nc.tensor.transpose(tp[:], gb[:, ts(e, 128)], idb[:])
                        nc.vector.tensor_copy(stg[:, e, :], tp[:])
                    nc.sync.dma_start(
                        embT_d[:].rearrange("e ki t -> ki e t")[:, :, ts(m, 128)],
                        stg[:],
                    )

                for g in range(4):
                    m_gate(g)
                    for m in range(5 * g, 5 * g + 5):
                        ph1_m(m)

            # ================= main pipeline pools =========================
            ltp = stack.enter_context(tc.tile_pool(name="ltp", bufs=2))
            sigp = stack.enter_context(tc.tile_pool(name="sigp", bufs=5))
            itp = stack.enter_context(tc.tile_pool(name="itp", bufs=2))
            fcp = stack.enter_context(tc.tile_pool(name="fcp", bufs=2))
            htp = stack.enter_context(tc.tile_pool(name="htp", bufs=2))
            pevp = stack.enter_context(tc.tile_pool(name="pevp", bufs=2))
            recp = stack.enter_context(tc.tile_pool(name="recp", bufs=3, space="PSUM"))
            up = stack.enter_context(tc.tile_pool(name="up", bufs=2, space="PSUM"))
            vp = stack.enter_context(tc.tile_pool(name="vp", bufs=2, space="PSUM"))
            tpp = stack.enter_context(tc.tile_pool(name="tpp", bufs=1, space="PSUM"))

            def load_lt(mt):
                lts = []
                for k in range(4):
                    lt = ltp.tile([128, 128], BF16, tag=f"lt{k}")
                    nc.scalar.dma_start(lt[:], embT_d[k, :, ts(mt, 128)])
                    lts.append(lt)
                return lts

            def u_steps(lts, mt, steps, gates=range(4)):
                """Produce stacked u' for steps (parities) of m-tile mt."""
                for s in steps:
                    for g in gates:
                        ps = up.tile([128, 512], F32, tag="up")
                        for k in range(4):
                            lhsT = lts[k][:, s * 64 : (s + 1) * 64]
                            nc.tensor.matmul(
                                ps[0:64, :],
                                lhsT=lhsT,
                                rhs=mcat[:, k, ts(2 * g, 512)],
                                start=(k == 0),
                                stop=(k == 3),
                                tile_position=(0, 0),
                            )
                            nc.tensor.matmul(
                                ps[64:128, :],
                                lhsT=lhsT,
                                rhs=mcat[:, k, ts(2 * g + 1, 512)],
                                start=(k == 0),
                                stop=(k == 3),
                                tile_position=(0, 64),
                            )
                        nc.vector.tensor_add(
                            u2_sb[mt % 2][s][:, ts(g, 512)], ps[:], ubias[:, ts(g, 512)]
                        )

            def rec_step(t):
                """One LSTM step: col-tiled gate matmuls + stacked elementwise."""
                mb = (t // 2) % 2
                upar = t % 2
                pmb = ((t - 1) // 2) % 2
                ppar = (t - 1) % 2
                sg = {}
                for g in GATE_ORDER:
                    ps = recp.tile([128, 512], F32, tag="rp")
                    if t > 0:
                        for k in range(8):
                            wsbX = wsbA if k < 4 else wsbB
                            lhsT = pairT[pmb][:, k, ppar * 64 : (ppar + 1) * 64]
                            nc.tensor.matmul(
                                ps[0:64, :],
                                lhsT=lhsT,
                                rhs=wsbX[:, k % 4, ts(2 * g, 512)],
                                start=(k == 0),
                                stop=False,
                                tile_position=(0, 0),
                            )
                            nc.tensor.matmul(
                                ps[64:128, :],
                                lhsT=lhsT,
                                rhs=wsbX[:, k % 4, ts(2 * g + 1, 512)],
                                start=(k == 0),
                                stop=False,
                                tile_position=(0, 64),
                            )
                    # fold +u into PSUM via identity-matmul: keeps the DVE
                    # out of the MM(o) -> act(o) -> ht critical chain
                    nc.tensor.matmul(
                        ps[:],
                        lhsT=idb[:],
                        rhs=u2_sb[mb][upar][:, ts(g, 512)],
                        start=(t == 0),
                        stop=True,
                    )
                    sgt = sigp.tile([128, 512], BF16, tag="sig")
                    nc.scalar.activation(sgt[:], ps[:], TANH if g == 3 else SIG)
                    sg[g] = sgt
                    if g == 1:  # ctilde, i, f done -> start the c-chain early
                        if t == 0:
                            nc.vector.tensor_mul(c_sb[:], sg[0][:], sg[3][:])
                        else:
                            it = itp.tile([128, 512], BF16, tag="it")
                            nc.vector.tensor_mul(it[:], sg[0][:], sg[3][:])
                            fc = fcp.tile([128, 512], BF16, tag="fc")
                            nc.vector.tensor_mul(fc[:], sg[1][:], c_sb[:])
                            nc.vector.tensor_add(c_sb[:], fc[:], it[:])
                ht = htp.tile([128, 512], BF16, tag="ht")
                nc.vector.tensor_mul(ht[:], sg[2][:], c_sb[:])
                return ht

            def transposes(t, ht):
                mb = (t // 2) % 2
                par = t % 2
                tp = tpp.tile([128, 4, 128], BF16, tag="tp")
                for q in range(4):
                    nc.tensor.transpose(tp[:, q, :], ht[:, ts(q, 128)], idb[:])
                # 2 strided copies (on ACT - DVE is chain-congested):
                # pairT[:, q+4l, par*64+b] <- tp[:, q, l*64+b]
                for l in range(2):
                    nc.scalar.activation(
                        pairT[mb][:, 4 * l : 4 * l + 4, par * 64 : (par + 1) * 64],
                        tp[:, :, ts(l, 64)],
                        COPY,
                    )

            def voc_chunks(vm, chunks):
                for n in chunks:
                    ps = vp.tile([128, NV], F32, tag="vpp")
                    for k in range(8):
                        ctX = ctA if k < 4 else ctB
                        nc.tensor.matmul(
                            ps[:],
                            lhsT=pairT[vm % 2][:, k, :],
                            rhs=ctX[:, k % 4, ts(n, NV)],
                            start=(k == 0),
                            stop=(k == 7),
                        )
                    pev = pevp.tile([128, NV], BF16, tag="pev")
                    nc.scalar.activation(pev[:], ps[:], COPY)
                    nc.gpsimd.dma_start(out_d[ts(vm, 128), ts(n, NV)], pev[:])

            # u(0) before the big late-weight DMAs so it isn't queued
            # behind 8MB of C^T transfer
            lts = load_lt(0)
            u_steps(lts, 0, [0, 1])

            # late vocab weights (transfers overlap the first iterations)
            late = stack.enter_context(tc.tile_pool(name="late", bufs=1))
            ctA = late.tile([128, 4, VS], BF16, tag="ctA")
            nc.sync.dma_start(
                ctA[:], CT_d[0 : H // 2, :].rearrange("(ko ki) n -> ki ko n", ki=128)
            )
            ctB = late.tile([128, 4, VS], BF16, tag="ctB")
            nc.sync.dma_start(
                ctB[:], CT_d[H // 2 :, :].rearrange("(ko ki) n -> ki ko n", ki=128)
            )

            for m in range(MT):
                lts = load_lt(m + 1) if m + 1 < MT else None
                ht = rec_step(2 * m)
                if lts is not None:
                    u_steps(lts, m + 1, [0], range(0, 2))
                transposes(2 * m, ht)
                if lts is not None:
                    u_steps(lts, m + 1, [0], range(2, 4))
                if m >= 1:
                    voc_chunks(m - 1, range(4, 8))
                ht = rec_step(2 * m + 1)
                if lts is not None:
                    u_steps(lts, m + 1, [1], range(0, 2))
                transposes(2 * m + 1, ht)
                if lts is not None:
                    u_steps(lts, m + 1, [1], range(2, 4))
                voc_chunks(m, range(4))
            voc_chunks(MT - 1, range(4, 8))

    nc.compile()
    return nc


def kernel(**inputs):
    import ml_dtypes

    BF = ml_dtypes.bfloat16

    captions = np.asarray(inputs["captions"])
    B_w = np.asarray(inputs["B_w"], dtype=np.float32)
    V_w = np.asarray(inputs["V_w"], dtype=np.float32)
    V_b = np.asarray(inputs["V_b"], dtype=np.float32)
    S_w = np.asarray(inputs["S_w"], dtype=np.float32)
    S_b = np.asarray(inputs["S_b"], dtype=np.float32)
    U_w = np.asarray(inputs["U_w"], dtype=np.float32)
    U_b = np.asarray(inputs["U_b"], dtype=np.float32)
    W_w = np.asarray(inputs["W_w"], dtype=np.float32)
    W_b = np.asarray(inputs["W_b"], dtype=np.float32)
    C_w = np.asarray(inputs["C_w"], dtype=np.float32)
    C_b = np.asarray(inputs["C_b"], dtype=np.float32)

    # --- host-side layout prep (weights only) ---
    # token order: tok = t*64 + b; pre-swizzled idx[p, m] = tok m*128 + p
    cap = np.ascontiguousarray(
        captions.T.reshape(TOK).reshape(MT, 128).T
    ).astype(np.int32)
    SgT = np.ascontiguousarray(S_w.transpose(0, 2, 1))
    UgT = np.ascontiguousarray(U_w.transpose(0, 2, 1))
    # gate-major columns: col = g*1024 + h
    Wmov = np.ascontiguousarray(W_w.transpose(2, 0, 1).reshape(H, 4 * H)).astype(BF)
    # gate bias chain, folded: ((V_b @ S^T + S_b) @ U^T + U_b) + W_b
    bs = np.einsum("gf,gof->go", V_b, S_w) + S_b
    bu = np.einsum("gf,ghf->gh", bs, U_w) + U_b
    gate_bias = (bu + W_b).reshape(4, 2, 512)
    # stacked bias: ub2[b + 64v, g*512 + h'] = gate_bias[g, v, h']
    ub2 = np.empty((128, 2 * H), np.float32)
    ub2[0:64, :] = np.broadcast_to(gate_bias[:, 0, :].reshape(2 * H), (64, 2 * H))
    ub2[64:128, :] = np.broadcast_to(gate_bias[:, 1, :].reshape(2 * H), (64, 2 * H))
    ub2 = np.ascontiguousarray(ub2).astype(BF)
    CT = np.ascontiguousarray(C_w.T)  # [H, V]

    nc = _build()

    in_maps = []
    for c in range(NCORES):
        in_maps.append(
            {
                "cap": cap,
                "Bw": B_w,
                "Vg": V_w,
                "SgT": SgT,
                "UgT": UgT,
                "Wmov": Wmov,
                "ubias2": ub2,
                "CT": np.ascontiguousarray(CT[:, c * VS : (c + 1) * VS]).astype(BF),
            }
        )

    global _last_in_maps
    _last_in_maps = in_maps

    res = run_bass_kernel_spmd(nc, in_maps, list(range(NCORES)))
    out = np.concatenate(
        [res.results[c]["out"].astype(np.float32) for c in range(NCORES)], axis=1
    )
    out += C_b[None, :]
    return out


_last_in_maps = None


# revision 33
# speedup vs baseline: 1.0476x; 1.0079x over previous
"""Trainium2 Bass kernel for nn_DecoderFactoredLSTM (v3: col-tiled bf16 pipeline).

Factored-LSTM decoder:
  emb = B_w[captions]                                   [B,T,E] -> tokens [T*B, E]
  u   = emb @ (V^T S^T U^T) + bias                      [T*B, 4H]   (gate pre-activations)
  recurrence over T=40 steps (LSTM, no tanh on c for h)
  out = hiddens @ C_w^T + C_b                           [T*B, V]

Sharding: recurrence + pre-projections replicated on all 8 cores; the
vocab projection (dominant FLOPs) sharded 8-way over vocab columns.

v3 = v2 (fused bf16 pipeline, SBUF-resident u/hT, interleaved fillers)
plus PE column-tiling: batch is 64 so gate/u matmuls only fill half the
128-wide stationary array; pairs of matmuls at tile_position (0,0) and
(0,64) run CONCURRENTLY, nearly halving recurrence+u PE time. Outputs
land stacked on psum partitions p = b + 64*v (v = h-half of the gate),
which also makes every elementwise op a full 128-partition [128,512] op.

Column layout (gate-major): col = g*1024 + 512*v + h'. W/M rhs chunk
(g,v) = cols [(2g+v)*512, ...). hT kept as pairT[buf][ki, ko, par*64+b]
(par = step parity in the m-tile); h-tile [b+64v, (q,h'')] transposes
q-slices into k-tiles ko = q + 4v.
"""

import sys
from contextlib import ExitStack

if "/opt/trn_rl_repo" not in sys.path:
    sys.path.insert(0, "/opt/trn_rl_repo")

import numpy as np

import concourse.bass as bass
import concourse.mybir as mybir
import concourse.tile as tile
from concourse import bacc
from concourse.bass import ts
from concourse.bass_utils import run_bass_kernel_spmd
from concourse.masks import make_identity

B, T, E, H, F, V = 64, 40, 512, 1024, 512, 32000
NCORES = 8
VS = V // NCORES  # 4000
TOK = T * B  # 2560
MT = TOK // 128  # 20
NV = VS // 8  # 500
F32 = mybir.dt.float32
F32R = mybir.dt.float32r
BF16 = mybir.dt.bfloat16
SIG = mybir.ActivationFunctionType.Sigmoid
TANH = mybir.ActivationFunctionType.Tanh
COPY = mybir.ActivationFunctionType.Copy

GATE_ORDER = [3, 0, 1, 2]  # ctilde first (tanh feeds the c-chain), then i, f, o


def _build():
    nc = bacc.Bacc(None, target_bir_lowering=False, debug=False)

    with tile.TileContext(nc) as tc:
        cap_d = nc.declare_dram_parameter("cap", [128, MT], mybir.dt.int32, isOutput=False)
        Bw_d = nc.declare_dram_parameter("Bw", [V, E], F32, isOutput=False)
        Vg_d = nc.declare_dram_parameter("Vg", [4, F, E], F32R, isOutput=False)
        SgT_d = nc.declare_dram_parameter("SgT", [4, F, F], F32R, isOutput=False)
        UgT_d = nc.declare_dram_parameter("UgT", [4, F, H], F32R, isOutput=False)
        W_d = nc.declare_dram_parameter("Wmov", [H, 4 * H], BF16, isOutput=False)
        ub_d = nc.declare_dram_parameter("ubias2", [128, 2 * H], BF16, isOutput=False)
        CT_d = nc.declare_dram_parameter("CT", [H, VS], BF16, isOutput=False)
        out_d = nc.declare_dram_parameter("out", [TOK, VS], BF16, isOutput=True)

        embT_d = nc.dram_tensor("embT", [4, 128, TOK], BF16)  # [e_outer, e_inner, tok]

        with ExitStack() as stack:
            pers = stack.enter_context(tc.tile_pool(name="pers", bufs=1))
            idb = pers.tile([128, 128], BF16, tag="idb")
            make_identity(nc, idb)
            # token indices first on the sync queue so gathers start early
            # (host pre-swizzles to [p, m] so the DMA is contiguous)
            idx_all = pers.tile([128, MT], mybir.dt.int32, tag="idx")
            nc.sync.dma_start(idx_all[:], cap_d[:])
            wsbA = pers.tile([128, 4, 4 * H], BF16, tag="wsbA")
            nc.sync.dma_start(
                wsbA[:], W_d[0 : H // 2, :].rearrange("(ko ki) n -> ki ko n", ki=128)
            )
            wsbB = pers.tile([128, 4, 4 * H], BF16, tag="wsbB")
            nc.sync.dma_start(
                wsbB[:], W_d[H // 2 :, :].rearrange("(ko ki) n -> ki ko n", ki=128)
            )
            ubias = pers.tile([128, 2 * H], BF16, tag="ubias")
            nc.sync.dma_start(ubias[:], ub_d[:])
            mcat = pers.tile([128, 4, 4 * H], BF16, tag="mcat")
            # u2_sb[mparity][stepparity]: stacked u' [b+64v, (g, h')]
            u2_sb = [
                [
                    pers.tile([128, 2 * H], BF16, tag=f"u{i}{s}", name=f"u{i}{s}")
                    for s in range(2)
                ]
                for i in range(2)
            ]
            pairT = [
                pers.tile([128, 8, 128], BF16, tag=f"pairT{i}", name=f"pairT{i}")
                for i in range(2)
            ]
            c_sb = pers.tile([128, 512], F32, tag="c")

            # ================= prologue: M = V^T S^T U^T, and embT =========
            # pipeline pools (created early: prologue shares up/tpp psum,
            # phase-1 gather/transpose work continues into the main loop)
            ph1b = stack.enter_context(tc.tile_pool(name="ph1b", bufs=2))
            ltp = stack.enter_context(tc.tile_pool(name="ltp", bufs=2))
            sigp = stack.enter_context(tc.tile_pool(name="sigp", bufs=4))
            itp = stack.enter_context(tc.tile_pool(name="itp", bufs=2))
            fcp = stack.enter_context(tc.tile_pool(name="fcp", bufs=2))
            htp = stack.enter_context(tc.tile_pool(name="htp", bufs=2))
            pevp = stack.enter_context(tc.tile_pool(name="pevp", bufs=2))
            recp = stack.enter_context(tc.tile_pool(name="recp", bufs=3, space="PSUM"))
            up = stack.enter_context(tc.tile_pool(name="up", bufs=2, space="PSUM"))
            vp = stack.enter_context(tc.tile_pool(name="vp", bufs=2, space="PSUM"))
            tpp = stack.enter_context(tc.tile_pool(name="tpp", bufs=1, space="PSUM"))

            def ph1_gather(m):
                g_t = ph1b.tile([128, E], F32, tag="gt")
                nc.gpsimd.indirect_dma_start(
                    out=g_t[:],
                    out_offset=None,
                    in_=Bw_d[:],
                    in_offset=bass.IndirectOffsetOnAxis(
                        ap=idx_all[:, m : m + 1], axis=0
                    ),
                )
                gt_tiles[m] = g_t

            def ph1_tp(m):
                gb = ph1b.tile([128, E], BF16, tag="gb")
                nc.vector.tensor_copy(gb[:], gt_tiles.pop(m)[:])
                tp = tpp.tile([128, 4, 128], BF16, tag="tp")
                for e in range(4):
                    nc.tensor.transpose(tp[:, e, :], gb[:, ts(e, 128)], idb[:])
                stg = ph1b.tile([128, 4, 128], BF16, tag="stg")
                nc.vector.tensor_copy(stg[:], tp[:])
                nc.sync.dma_start(
                    embT_d[:].rearrange("e ki t -> ki e t")[:, :, ts(m, 128)],
                    stg[:],
                )

            gt_tiles = {}

            with (
                tc.tile_pool(name="ph2a", bufs=2) as ph2a,
                tc.tile_pool(name="ph2u", bufs=1) as ph2u,
                tc.tile_pool(name="ph2b", bufs=1) as ph2b,
            ):

                def m_gate(g):
                    vg = ph2a.tile([128, 4, E], F32R, tag="vg")
                    nc.scalar.dma_start(
                        vg[:], Vg_d[g].rearrange("(ko ki) e -> ki ko e", ki=128)
                    )
                    sgT = ph2a.tile([128, 4, F], F32R, tag="sgT")
                    nc.scalar.dma_start(
                        sgT[:], SgT_d[g].rearrange("(ko ki) f -> ki ko f", ki=128)
                    )
                    ugT = ph2u.tile([128, 4, H], F32R, tag="ugT")
                    nc.scalar.dma_start(
                        ugT[:], UgT_d[g].rearrange("(ko ki) f -> ki ko f", ki=128)
                    )
                    # PT[f', e] = sum_f S[f',f] V[f,e]
                    pt = ph2b.tile([128, 4, E], F32R, tag="pt")
                    for fp in range(4):
                        ps = up.tile([128, E], F32, tag="up")
                        for k in range(4):
                            nc.tensor.matmul(
                                ps[:],
                                lhsT=sgT[:, k, ts(fp, 128)],
                                rhs=vg[:, k, :],
                                start=(k == 0),
                                stop=(k == 3),
                            )
                        nc.vector.tensor_copy(pt[:, fp, :], ps[:])
                    # M[e, col], col = g*1024 + nh*512 + h'
                    for e_t in range(4):
                        for nh in range(2):
                            ps2 = up.tile([128, 512], F32, tag="up")
                            for k in range(4):
                                nc.tensor.matmul(
                                    ps2[:],
                                    lhsT=pt[:, k, ts(e_t, 128)],
                                    rhs=ugT[:, k, ts(nh, 512)],
                                    start=(k == 0),
                                    stop=(k == 3),
                                )
                            nc.vector.tensor_copy(
                                mcat[:, e_t, g * 1024 + nh * 512 : g * 1024 + (nh + 1) * 512],
                                ps2[:],
                            )

                ph1_gather(0)
                ph1_gather(1)
                for g in range(4):
                    m_gate(g)
                    for m in (2 * g, 2 * g + 1):
                        ph1_tp(m)
                        ph1_gather(m + 2)
                # gathers 0..9 issued, embT tiles 0..7 written; tiles 8..19
                # are produced inside the main loop (one per iteration)

            def load_lt(mt):
                lts = []
                for k in range(4):
                    lt = ltp.tile([128, 128], BF16, tag=f"lt{k}")
                    nc.scalar.dma_start(lt[:], embT_d[k, :, ts(mt, 128)])
                    lts.append(lt)
                return lts

            def u_steps(lts, mt, steps, gates=range(4)):
                """Produce stacked u' for steps (parities) of m-tile mt."""
                for s in steps:
                    for g in gates:
                        ps = up.tile([128, 512], F32, tag="up")
                        for k in range(4):
                            lhsT = lts[k][:, s * 64 : (s + 1) * 64]
                            nc.tensor.matmul(
                                ps[0:64, :],
                                lhsT=lhsT,
                                rhs=mcat[:, k, ts(2 * g, 512)],
                                start=(k == 0),
                                stop=(k == 3),
                                tile_position=(0, 0),
                            )
                            nc.tensor.matmul(
                                ps[64:128, :],
                                lhsT=lhsT,
                                rhs=mcat[:, k, ts(2 * g + 1, 512)],
                                start=(k == 0),
                                stop=(k == 3),
                                tile_position=(0, 64),
                            )
                        nc.vector.tensor_add(
                            u2_sb[mt % 2][s][:, ts(g, 512)], ps[:], ubias[:, ts(g, 512)]
                        )

            def rec_step(t):
                """One LSTM step: col-tiled gate matmuls + stacked elementwise."""
                mb = (t // 2) % 2
                upar = t % 2
                pmb = ((t - 1) // 2) % 2
                ppar = (t - 1) % 2
                sg = {}
                for g in GATE_ORDER:
                    ps = recp.tile([128, 512], F32, tag="rp")
                    if t > 0:
                        for k in range(8):
                            wsbX = wsbA if k < 4 else wsbB
                            lhsT = pairT[pmb][:, k, ppar * 64 : (ppar + 1) * 64]
                            nc.tensor.matmul(
                                ps[0:64, :],
                                lhsT=lhsT,
                                rhs=wsbX[:, k % 4, ts(2 * g, 512)],
                                start=(k == 0),
                                stop=False,
                                tile_position=(0, 0),
                            )
                            nc.tensor.matmul(
                                ps[64:128, :],
                                lhsT=lhsT,
                                rhs=wsbX[:, k % 4, ts(2 * g + 1, 512)],
                                start=(k == 0),
                                stop=False,
                                tile_position=(0, 64),
                            )
                    # fold +u into PSUM via identity-matmul: keeps the DVE
                    # out of the MM(o) -> act(o) -> ht critical chain
                    nc.tensor.matmul(
                        ps[:],
                        lhsT=idb[:],
                        rhs=u2_sb[mb][upar][:, ts(g, 512)],
                        start=(t == 0),
                        stop=True,
                    )
                    sgt = sigp.tile([128, 512], BF16, tag="sig")
                    nc.scalar.activation(sgt[:], ps[:], TANH if g == 3 else SIG)
                    sg[g] = sgt
                    if g == 1:  # ctilde, i, f done -> start the c-chain early
                        if t == 0:
                            nc.vector.tensor_mul(c_sb[:], sg[0][:], sg[3][:])
                        else:
                            it = itp.tile([128, 512], BF16, tag="it")
                            nc.vector.tensor_mul(it[:], sg[0][:], sg[3][:])
                            fc = fcp.tile([128, 512], BF16, tag="fc")
                            nc.vector.tensor_mul(fc[:], sg[1][:], c_sb[:])
                            nc.vector.tensor_add(c_sb[:], fc[:], it[:])
                ht = htp.tile([128, 512], BF16, tag="ht")
                nc.vector.tensor_mul(ht[:], sg[2][:], c_sb[:])
                return ht

            def transposes(t, ht):
                mb = (t // 2) % 2
                par = t % 2
                tp = tpp.tile([128, 4, 128], BF16, tag="tp")
                for q in range(4):
                    nc.tensor.transpose(tp[:, q, :], ht[:, ts(q, 128)], idb[:])
                # 2 strided copies (on ACT - DVE is chain-congested):
                # pairT[:, q+4l, par*64+b] <- tp[:, q, l*64+b]
                for l in range(2):
                    nc.scalar.activation(
                        pairT[mb][:, 4 * l : 4 * l + 4, par * 64 : (par + 1) * 64],
                        tp[:, :, ts(l, 64)],
                        COPY,
                    )

            def voc_chunks(vm, chunks):
                for n in chunks:
                    ps = vp.tile([128, NV], F32, tag="vpp")
                    for k in range(8):
                        ctX = ctA if k < 4 else ctB
                        nc.tensor.matmul(
                            ps[:],
                            lhsT=pairT[vm % 2][:, k, :],
                            rhs=ctX[:, k % 4, ts(n, NV)],
                            start=(k == 0),
                            stop=(k == 7),
                        )
                    pev = pevp.tile([128, NV], BF16, tag="pev")
                    nc.scalar.activation(pev[:], ps[:], COPY)
                    nc.gpsimd.dma_start(out_d[ts(vm, 128), ts(n, NV)], pev[:])

            # u(0) before the big late-weight DMAs so it isn't queued
            # behind 8MB of C^T transfer
            lts = load_lt(0)
            u_steps(lts, 0, [0, 1])

            # late vocab weights (transfers overlap the first iterations)
            late = stack.enter_context(tc.tile_pool(name="late", bufs=1))
            ctA = late.tile([128, 4, VS], BF16, tag="ctA")
            nc.sync.dma_start(
                ctA[:], CT_d[0 : H // 2, :].rearrange("(ko ki) n -> ki ko n", ki=128)
            )
            ctB = late.tile([128, 4, VS], BF16, tag="ctB")
            nc.sync.dma_start(
                ctB[:], CT_d[H // 2 :, :].rearrange("(ko ki) n -> ki ko n", ki=128)
            )

            for m in range(MT):
                lts = load_lt(m + 1) if m + 1 < MT else None
                if m + 10 < MT:
                    ph1_gather(m + 10)
                ht = rec_step(2 * m)
                if lts is not None:
                    u_steps(lts, m + 1, [0], range(0, 2))
                transposes(2 * m, ht)
                if lts is not None:
                    u_steps(lts, m + 1, [0], range(2, 4))
                if m >= 1:
                    voc_chunks(m - 1, range(4, 8))
                if m + 8 < MT:
                    ph1_tp(m + 8)
                ht = rec_step(2 * m + 1)
                if lts is not None:
                    u_steps(lts, m + 1, [1], range(0, 2))
                transposes(2 * m + 1, ht)
                if lts is not None:
                    u_steps(lts, m + 1, [1], range(2, 4))
                voc_chunks(m, range(4))
            voc_chunks(MT - 1, range(4, 8))

    nc.compile()
    return nc


def kernel(**inputs):
    import ml_dtypes

    BF = ml_dtypes.bfloat16

    captions = np.asarray(inputs["captions"])
    B_w = np.asarray(inputs["B_w"], dtype=np.float32)
    V_w = np.asarray(inputs["V_w"], dtype=np.float32)
    V_b = np.asarray(inputs["V_b"], dtype=np.float32)
    S_w = np.asarray(inputs["S_w"], dtype=np.float32)
    S_b = np.asarray(inputs["S_b"], dtype=np.float32)
    U_w = np.asarray(inputs["U_w"], dtype=np.float32)
    U_b = np.asarray(inputs["U_b"], dtype=np.float32)
    W_w = np.asarray(inputs["W_w"], dtype=np.float32)
    W_b = np.asarray(inputs["W_b"], dtype=np.float32)
    C_w = np.asarray(inputs["C_w"], dtype=np.float32)
    C_b = np.asarray(inputs["C_b"], dtype=np.float32)

    # --- host-side layout prep (weights only) ---
    # token order: tok = t*64 + b; pre-swizzled idx[p, m] = tok m*128 + p
    cap = np.ascontiguousarray(
        captions.T.reshape(TOK).reshape(MT, 128).T
    ).astype(np.int32)
    SgT = np.ascontiguousarray(S_w.transpose(0, 2, 1))
    UgT = np.ascontiguousarray(U_w.transpose(0, 2, 1))
    # gate-major columns: col = g*1024 + h
    Wmov = np.ascontiguousarray(W_w.transpose(2, 0, 1).reshape(H, 4 * H)).astype(BF)
    # gate bias chain, folded: ((V_b @ S^T + S_b) @ U^T + U_b) + W_b
    bs = np.einsum("gf,gof->go", V_b, S_w) + S_b
    bu = np.einsum("gf,ghf->gh", bs, U_w) + U_b
    gate_bias = (bu + W_b).reshape(4, 2, 512)
    # stacked bias: ub2[b + 64v, g*512 + h'] = gate_bias[g, v, h']
    ub2 = np.empty((128, 2 * H), np.float32)
    ub2[0:64, :] = np.broadcast_to(gate_bias[:, 0, :].reshape(2 * H), (64, 2 * H))
    ub2[64:128, :] = np.broadcast_to(gate_bias[:, 1, :].reshape(2 * H), (64, 2 * H))
    ub2 = np.ascontiguousarray(ub2).astype(BF)
    CT = np.ascontiguousarray(C_w.T)  # [H, V]

    nc = _build()

    in_maps = []
    for c in range(NCORES):
        in_maps.append(
            {
                "cap": cap,
                "Bw": B_w,
                "Vg": V_w,
                "SgT": SgT,
                "UgT": UgT,
                "Wmov": Wmov,
                "ubias2": ub2,
                "CT": np.ascontiguousarray(CT[:, c * VS : (c + 1) * VS]).astype(BF),
            }
        )

    global _last_in_maps
    _last_in_maps = in_maps

    res = run_bass_kernel_spmd(nc, in_maps, list(range(NCORES)))
    out = np.concatenate(
        [res.results[c]["out"].astype(np.float32) for c in range(NCORES)], axis=1
    )
    out += C_b[None, :]
    return out


_last_in_maps = None
